# revision 15
# baseline (speedup 1.0000x reference)
"""Chain-CRF negative log-likelihood on 8 Trainium2 NeuronCores (Bass/Tile).

Strategy (pure data parallelism, batch 512 -> 64 per core), v2:
  Scaled-exp-space forward algorithm, meet-in-the-middle: a forward chain
  (t = 0..127) and a backward beta chain (tau = 255..128) advance together,
  ONE matmul + ONE DVE multiply per round, via a 112-partition block
  layout sharing a single [112, 112] stationary matrix:

    rows  0..47   forward state p_t           (A block: exp(lt), w col 48)
    row   48      selected-answer carrier     (E-slab row 48 = 1{L_b == r})
    row   49      constant 1                  (E-slab row 49 = 1)
    row   50      injection carrier           (E-slab row 50 = 1{L_b == 255-r})
    row   51      answer accumulator          (E-slab row 51 = 1)
    rows 64..111  backward state u_tau        (B block: exp(lt)^T, inj row 50)

  Per-sequence termination is free: psum row 48 = sum_i tend[i] p_{t}[i]
  (the linear partition value if the sequence ends here); the E-slab row 48
  selects it exactly at r == L_b and row 51 accumulates it. Backward
  injections (beta_{L-1} = tend) enter through the carrier row 50 whose
  per-round value rides in the E-slab — so variable lengths cost zero
  extra instructions. Numerical scaling is a CONSTANT per-step factor
  e^-BIAS folded into the Exp activation bias; the exact compensation
  L_b * BIAS is folded into the host-prepared emission sums.

  Gold path score: transition counts matmul (host-built integer count
  matrix x device exp->log values) + device emission sum, as in v1.
  Host work remains layout transforms + integer index/mask/count prep +
  emission value selection.
"""

import os

os.environ.setdefault("NEURON_CC_FLAGS", "")

import numpy as np
from contextlib import ExitStack

import concourse.bass as bass
import concourse.tile as tile
from concourse import bacc, mybir
from concourse.bass_utils import run_bass_kernel_spmd

# ---- problem constants (hardcoded per contract) ----
B = 512
M = 256
T = 48          # n_tags
ROOT = 46
END = 47
NC = 8
BL = B // NC    # 64 sequences per core
R = 128         # rounds: fwd t = 0..127, bwd tau = 255..128
P = 112         # partition span of the block layout
BIAS = 4.36     # per-step constant renorm, e^-BIAS (compensated exactly)
LN2_12 = 12.0 * float(np.log(2.0))

F32 = mybir.dt.float32
AF = mybir.ActivationFunctionType
ALU = mybir.AluOpType

_PROGRAM = None


def _build_program():
    nc = bacc.Bacc(
        "TRN2",
        target_bir_lowering=False,
        debug=False,
        enable_asserts=False,
        num_devices=NC,
    )

    ftg = nc.dram_tensor("ftg", [T, 2 * R * BL], F32, kind="ExternalInput").ap()
    lt = nc.dram_tensor("lt", [T, T], F32, kind="ExternalInput").ap()
    ltT = nc.dram_tensor("ltT", [T, T], F32, kind="ExternalInput").ap()
    ltflat = nc.dram_tensor("ltflat", [128, 18], F32, kind="ExternalInput").ap()
    aux = nc.dram_tensor("aux", [4, R * BL], F32, kind="ExternalInput").ap()
    auxinit = nc.dram_tensor("auxinit", [2, BL], F32, kind="ExternalInput").ap()
    lhaux = nc.dram_tensor("lhaux", [4, 3 * P], F32, kind="ExternalInput").ap()
    emitv = nc.dram_tensor("emitv", [BL, M], F32, kind="ExternalInput").ap()
    cmat = nc.dram_tensor("cmat", [128, 18 * BL], F32, kind="ExternalInput").ap()
    out = nc.dram_tensor("out", [BL, 1], F32, kind="ExternalOutput").ap()

    with tile.TileContext(nc) as tc, ExitStack() as ctx:
        _emit_body(ctx, tc, ftg, lt, ltT, ltflat, aux, auxinit, lhaux, emitv,
                   cmat, out)
    nc.finalize()
    return nc


def _emit_body(ctx, tc, ftg, lt, ltT, ltflat, aux, auxinit, lhaux, emitv, cmat,
               out):
    nc = tc.nc

    const = ctx.enter_context(tc.tile_pool(name="const", bufs=1))
    raws = ctx.enter_context(tc.tile_pool(name="raws", bufs=2))
    states = ctx.enter_context(tc.tile_pool(name="states", bufs=4))
    endp = ctx.enter_context(tc.tile_pool(name="endp", bufs=2))
    ps_pool = ctx.enter_context(tc.tile_pool(name="ps", bufs=2, space="PSUM"))
    zw_pool = ctx.enter_context(tc.tile_pool(name="zw", bufs=1, space="PSUM"))
    pair_pool = ctx.enter_context(tc.tile_pool(name="pairps", bufs=1, space="PSUM"))

    # ---------------- small constants ----------------
    ones11 = const.tile([1, 2], F32)
    nc.gpsimd.memset(ones11[:], 1.0)
    onescol = const.tile([T, 1], F32)
    nc.gpsimd.memset(onescol[:], 1.0)
    mbiasF = const.tile([T, 1], F32)
    nc.gpsimd.memset(mbiasF[:], -BIAS)
    mbiasB = const.tile([P, 1], F32)
    nc.gpsimd.memset(mbiasB[64:112, :], -BIAS)

    # SP/HWDGE queue: everything gating the first rounds, smallest first
    lt_sb = const.tile([T, T], F32)
    nc.sync.dma_start(lt_sb[:], lt[:])
    ltT_sb = const.tile([P, T], F32)   # partitions 64..111: same-start Exp
    nc.sync.dma_start(ltT_sb[64:112, :], ltT[:])
    rootcol = const.tile([T, 1], F32)
    nc.sync.dma_start(rootcol[:], lt[ROOT:ROOT + 1, :].rearrange("a b -> b a"))
    biasvec = const.tile([T, 1], F32)
    nc.vector.tensor_scalar_add(biasvec[:], rootcol[:], -BIAS)

    # ---------------- stationary matrices [112, 112] ----------------
    # col j (output partition): 0..47 fwd T^T; 48 w (tend); 49 const-1;
    # 50 carrier; 51 accumulator; 64..111 bwd T (+ inj via row 50).
    lhsT_init = const.tile([P, P], F32)
    lhsT_main = const.tile([P, P], F32)
    lhsT_fin = const.tile([P, P], F32)
    nc.gpsimd.memset(lhsT_init[:], 0.0)
    nc.gpsimd.memset(lhsT_main[:], 0.0)
    nc.gpsimd.memset(lhsT_fin[:], 0.0)
    # rows 48..51 (selector routing, const-1, tend injection row) are tiny
    # host-baked patterns: one DMA per tile
    lhts = (lhsT_init, lhsT_main, lhsT_fin)
    for k, lh in enumerate(lhts):
        nc.sync.dma_start(lh[48:52, :], lhaux[:, k * P:(k + 1) * P])
    # initial state: zeros except row 49 = 1 and row 50 = inject(L=256)
    st0 = states.tile([P, BL], F32, tag="st")
    nc.gpsimd.memset(st0[:], 0.0)
    nc.sync.dma_start(st0[49:51, :], auxinit[:])
    for lh in (lhsT_init, lhsT_main):
        nc.scalar.activation(lh[0:T, 0:T], lt_sb[:], AF.Exp)          # A block
        nc.scalar.activation(lh[0:T, 48:49], lt_sb[:, END:END + 1], AF.Exp)  # w col
    for lh in lhts:
        nc.scalar.activation(lh[64:112, 64:112], ltT_sb[64:112, :], AF.Exp)  # B block

    # ---------------- E slab [112, 128*64] ----------------
    # fwd rows 0..47 (t = round) via SP/HWDGE, bwd rows 64..111
    # (tau = 255 - round, host pre-reversed) via Pool/SWDGE — parallel DMA
    # paths, fwd/bwd interleaved so round k's blocks arrive in step.
    slab = const.tile([P, R * BL], F32)
    pieces = [(0, 512), (512, 512), (1024, 1024), (2048, 2048), (4096, 4096)]
    chunks = [(0, 512), (512, 1536), (2048, 2048), (4096, 4096)]
    nc.gpsimd.memset(slab[32:64, 0:512], 0.0)
    nc.gpsimd.dma_start(slab[48:52, 0:512], aux[:, 0:512])
    for pi, (off, ln) in enumerate(pieces):
        rawf = raws.tile([T, 4096], F32, tag="rawf")
        nc.sync.dma_start(rawf[:, 0:ln], ftg[:, off:off + ln])
        rawb = raws.tile([P, 4096], F32, tag="rawb")
        nc.gpsimd.dma_start(rawb[64:112, 0:ln], ftg[:, 8192 + off:8192 + off + ln])
        if off == 0:
            # block 0 carries the ROOT prior in its bias; remainder plain
            nc.scalar.activation(slab[0:T, 0:BL], rawf[:, 0:BL], AF.Exp,
                                 bias=biasvec[:])
            nc.scalar.activation(slab[0:T, BL:ln], rawf[:, BL:ln], AF.Exp,
                                 bias=mbiasF[:])
        else:
            nc.scalar.activation(slab[0:T, off:off + ln], rawf[:, 0:ln], AF.Exp,
                                 bias=mbiasF[:])
        nc.scalar.activation(slab[64:112, off:off + ln], rawb[64:112, 0:ln],
                             AF.Exp, bias=mbiasB[64:112, :])
        if pi + 1 < len(chunks):
            coff, cln = chunks[pi + 1]
            nc.gpsimd.memset(slab[32:64, coff:coff + cln], 0.0)
            nc.gpsimd.dma_start(slab[48:52, coff:coff + cln], aux[:, coff:coff + cln])

    # late constants (gold score path) on the Pool/SWDGE queue
    ltsb = const.tile([128, 18], F32)
    nc.gpsimd.dma_start(ltsb[:], ltflat[:])
    cmat_sb = const.tile([128, 18 * BL], F32)
    nc.gpsimd.dma_start(cmat_sb[:], cmat[:])
    emitv_sb = const.tile([BL, M], F32)
    nc.gpsimd.dma_start(emitv_sb[:], emitv[:])

    # prefetch the Ln act table once the Exps are done (reads last exp'd cell
    # to order after them on the Act queue)
    lnjunk = const.tile([1, 1], F32)
    nc.scalar.activation(lnjunk[:], slab[0:1, R * BL - 1:R * BL], AF.Ln)

    # ---------------- gold score (independent of the scan) ----------------
    emitsum = const.tile([BL, 1], F32)
    nc.vector.tensor_reduce(emitsum[:], emitv_sb[:], axis=mybir.AxisListType.X,
                            op=ALU.add)
    pair_ps = pair_pool.tile([BL, 1], F32, space="PSUM")
    for k in range(18):
        nc.tensor.matmul(
            out=pair_ps[:],
            lhsT=cmat_sb[:, k * BL:(k + 1) * BL],
            rhs=ltsb[:, k:k + 1],
            start=(k == 0),
            stop=(k == 17),
        )

    # ---------------- the scan: 128 rounds + final matmul ----------------
    prev = st0
    ps = None
    ps128 = None
    for i in range(R + 1):
        col = i % 8
        if col == 0:
            ps = ps_pool.tile([P, 512], F32, space="PSUM")
        lh = lhsT_init if i == 0 else (lhsT_fin if i == R else lhsT_main)
        nc.tensor.matmul(out=ps[:, col * BL:(col + 1) * BL], lhsT=lh[:],
                         rhs=prev[:], start=True, stop=True)
        if i < R:
            cur = states.tile([P, BL], F32, tag="st")
            eng = nc.gpsimd if os.environ.get("K_POOL_TT") else nc.vector
            eng.tensor_tensor(out=cur[:], in0=ps[:, col * BL:(col + 1) * BL],
                              in1=slab[:, i * BL:(i + 1) * BL], op=ALU.mult)
            prev = cur
        else:
            ps128 = ps[:, col * BL:(col + 1) * BL]

    # ---------------- epilogue ----------------
    # z (long sequences): zz = p_127 o beta_127, then column-sum via matmul.
    zz = endp.tile([T, BL], F32)
    nc.vector.tensor_tensor(out=zz[:], in0=prev[0:T, :], in1=ps128[64:112],
                            op=ALU.mult)
    # wsel (short sequences): accumulated at psum row 32 by lhsT_fin.
    wrow = endp.tile([1, BL], F32)
    nc.vector.tensor_copy(wrow[:], ps128[32:33])
    # part0[b] = z[b] + wsel[b] (disjoint support), as a [64,1] column
    zw = zw_pool.tile([BL, 1], F32, space="PSUM")
    nc.tensor.matmul(out=zw[:], lhsT=zz[:], rhs=onescol[:], start=True, stop=False)
    nc.tensor.matmul(out=zw[:], lhsT=wrow[:], rhs=ones11[0:1, 0:1], start=False,
                     stop=True)
    zws = endp.tile([BL, 1], F32)
    nc.vector.tensor_scalar(out=zws[:], in0=zw[:], scalar1=1e-37,
                            scalar2=2.0 ** -12, op0=ALU.max, op1=ALU.mult)
    lnp = endp.tile([BL, 1], F32)
    nc.scalar.activation(lnp[:], zws[:], AF.Ln)
    # nll = ln(part0) - pair - emitsum   (scale/BIAS comp folded into emitv)
    n1 = endp.tile([BL, 1], F32)
    nc.vector.tensor_tensor(out=n1[:], in0=lnp[:], in1=pair_ps[:], op=ALU.subtract)
    nll = endp.tile([BL, 1], F32)
    nc.vector.tensor_tensor(out=nll[:], in0=n1[:], in1=emitsum[:], op=ALU.subtract)
    nc.sync.dma_start(out[:], nll[:])


# ---------------- host side ----------------

def _host_prep_core(feats_c, tags_c, lengths_c):
    """Host work: layout transforms + integer index/mask/count prep +
    emission value selection (no float arithmetic beyond constant offsets)."""
    L = lengths_c.astype(np.int64)
    tg = tags_c.astype(np.int64)
    bidx = np.arange(BL)
    tidx = np.arange(M)

    ft = np.ascontiguousarray(feats_c.transpose(2, 1, 0))          # [48, 256, 64]
    ftg = np.empty((T, 2 * R * BL), np.float32)
    ftg[:, :R * BL] = ft[:, 0:R, :].reshape(T, R * BL)
    ftg[:, R * BL:] = ft[:, :R - 1:-1, :].reshape(T, R * BL)       # tau = 255-i

    rr = np.arange(R)
    aux = np.zeros((4, R, BL), np.float32)
    aux[0] = (L[None, :] == rr[:, None])           # row 48: select w at r == L
    aux[1] = 1.0                                   # row 49: const one
    aux[2] = (L[None, :] == 255 - rr[:, None])     # row 50: inject at L == 255-r
    aux[3] = 1.0                                   # row 51: accumulator carry

    auxinit = np.zeros((2, BL), np.float32)
    auxinit[0] = 1.0
    auxinit[1] = (L == 256)

    mask = tidx[None, :] < L[:, None]
    emitv = np.where(mask, feats_c[bidx[:, None], tidx[None, :], tg], 0.0)
    emitv = np.ascontiguousarray(emitv, np.float32)
    # fold the exact renorm/Ln-scale compensation into the emission sum
    emitv[:, 0] -= (L * BIAS + LN2_12).astype(np.float32)

    cfull = np.zeros((T * T, BL), np.float32)
    prev = tg[:, :-1]
    nxt = tg[:, 1:]
    pmask = (tidx[1:][None, :] < L[:, None])
    pidx = (prev * T + nxt)
    np.add.at(cfull, (pidx[pmask], np.broadcast_to(bidx[:, None], pidx.shape)[pmask]), 1.0)
    np.add.at(cfull, (ROOT * T + tg[:, 0], bidx), 1.0)
    last = tg[bidx, L - 1]
    np.add.at(cfull, (last * T + END, bidx), 1.0)
    cmat = np.ascontiguousarray(
        cfull.reshape(18, 128, BL).transpose(1, 0, 2)).reshape(128, 18 * BL)

    return {
        "ftg": ftg,
        "aux": aux.reshape(4, R * BL),
        "auxinit": auxinit,
        "emitv": emitv,
        "cmat": cmat,
    }


def kernel(feats, tags, lengths, log_transitions):
    global _PROGRAM
    feats = np.asarray(feats, np.float32)
    tags = np.asarray(tags)
    lengths = np.asarray(lengths)
    lt = np.asarray(log_transitions, np.float32)
    ltT = np.ascontiguousarray(lt.T)
    ltflat = np.ascontiguousarray(lt.reshape(-1).reshape(18, 128).T)

    # rows 48..51 of the three stationary matrices (48=w->acc routing,
    # 49=const-1 + carrier feed, 50=tend injection, 51=acc carry)
    tend = np.exp(lt[:, END].astype(np.float64)).astype(np.float32)
    lhaux = np.zeros((4, 3, P), np.float32)
    for k in range(3):
        lhaux[1, k, 49] = 1.0   # psum[49] = state[49] (const 1)
        lhaux[1, k, 50] = 1.0   # psum[50] = state[49] (carrier feed)
        lhaux[2, k, 64:112] = tend   # injection into bwd block
    lhaux[1, 0, 0:T] = 1.0      # init: seed fwd state with ones
    lhaux[0, 1, 51] = 1.0       # main: acc += selected w
    lhaux[3, 1, 51] = 1.0       # main: acc carry
    lhaux[0, 2, 32] = 1.0       # fin: route (sel + acc) to psum row 32
    lhaux[3, 2, 32] = 1.0
    lhaux = lhaux.reshape(4, 3 * P)

    in_maps = []
    for c in range(NC):
        sl = slice(c * BL, (c + 1) * BL)
        m = _host_prep_core(feats[sl], tags[sl], lengths[sl])
        m["lt"] = lt
        m["ltT"] = ltT
        m["ltflat"] = ltflat
        m["lhaux"] = lhaux
        in_maps.append(m)

    if _PROGRAM is None:
        _PROGRAM = _build_program()

    res = run_bass_kernel_spmd(_PROGRAM, in_maps, core_ids=list(range(NC)))
    return np.concatenate([r["out"].reshape(BL) for r in res.results])


if __name__ == "__main__":
    rng = np.random.default_rng(0)
    feats = rng.standard_normal((B, M, T)).astype(np.float32)
    tags = rng.integers(0, ROOT, (B, M)).astype(np.int32)
    lengths = rng.integers(1, M + 1, (B,)).astype(np.int32)
    std = (2.0 / (T + T)) ** 0.5
    lt = (rng.standard_normal((T, T)) * std).astype(np.float32)
    lt[:, ROOT] = -10000.0
    lt[END, :] = -10000.0
    out = kernel(feats, tags, lengths, lt)
    print(out[:8], out.shape, out.dtype)


# revision 17
# speedup vs baseline: 1.0044x; 1.0044x over previous
"""Chain-CRF negative log-likelihood on 8 Trainium2 NeuronCores (Bass/Tile).

Strategy (pure data parallelism, batch 512 -> 64 per core), v2:
  Scaled-exp-space forward algorithm, meet-in-the-middle: a forward chain
  (t = 0..127) and a backward beta chain (tau = 255..128) advance together,
  ONE matmul + ONE DVE multiply per round, via a 112-partition block
  layout sharing a single [112, 112] stationary matrix:

    rows  0..47   forward state p_t           (A block: exp(lt), w col 48)
    row   48      selected-answer carrier     (E-slab row 48 = 1{L_b == r})
    row   49      constant 1                  (E-slab row 49 = 1)
    row   50      injection carrier           (E-slab row 50 = 1{L_b == 255-r})
    row   51      answer accumulator          (E-slab row 51 = 1)
    rows 64..111  backward state u_tau        (B block: exp(lt)^T, inj row 50)

  Per-sequence termination is free: psum row 48 = sum_i tend[i] p_{t}[i]
  (the linear partition value if the sequence ends here); the E-slab row 48
  selects it exactly at r == L_b and row 51 accumulates it. Backward
  injections (beta_{L-1} = tend) enter through the carrier row 50 whose
  per-round value rides in the E-slab — so variable lengths cost zero
  extra instructions. Numerical scaling is a CONSTANT per-step factor
  e^-BIAS folded into the Exp activation bias; the exact compensation
  L_b * BIAS is folded into the host-prepared emission sums.

  Gold path score: transition counts matmul (host-built integer count
  matrix x device exp->log values) + device emission sum, as in v1.
  Host work remains layout transforms + integer index/mask/count prep +
  emission value selection.
"""

import os

os.environ.setdefault("NEURON_CC_FLAGS", "")

import numpy as np
from contextlib import ExitStack

import concourse.bass as bass
import concourse.tile as tile
from concourse import bacc, mybir
from concourse.bass_utils import run_bass_kernel_spmd

# ---- problem constants (hardcoded per contract) ----
B = 512
M = 256
T = 48          # n_tags
ROOT = 46
END = 47
NC = 8
BL = B // NC    # 64 sequences per core
R = 128         # rounds: fwd t = 0..127, bwd tau = 255..128
P = 112         # partition span of the block layout
BIAS = 4.36     # per-step constant renorm, e^-BIAS (compensated exactly)
LN2_12 = 12.0 * float(np.log(2.0))

F32 = mybir.dt.float32
AF = mybir.ActivationFunctionType
ALU = mybir.AluOpType

_PROGRAM = None


def _build_program():
    nc = bacc.Bacc(
        "TRN2",
        target_bir_lowering=False,
        debug=False,
        enable_asserts=False,
        num_devices=NC,
    )

    ftg = nc.dram_tensor("ftg", [T, 2 * R * BL], F32, kind="ExternalInput").ap()
    lt = nc.dram_tensor("lt", [T, T], F32, kind="ExternalInput").ap()
    ltT = nc.dram_tensor("ltT", [T, T], F32, kind="ExternalInput").ap()
    ltflat = nc.dram_tensor("ltflat", [128, 18], F32, kind="ExternalInput").ap()
    aux = nc.dram_tensor("aux", [4, R * BL], F32, kind="ExternalInput").ap()
    auxinit = nc.dram_tensor("auxinit", [2, BL], F32, kind="ExternalInput").ap()
    lhaux = nc.dram_tensor("lhaux", [4, 3 * P], F32, kind="ExternalInput").ap()
    emitv = nc.dram_tensor("emitv", [BL, M], F32, kind="ExternalInput").ap()
    cmat = nc.dram_tensor("cmat", [128, 18 * BL], F32, kind="ExternalInput").ap()
    out = nc.dram_tensor("out", [BL, 1], F32, kind="ExternalOutput").ap()

    with tile.TileContext(nc) as tc, ExitStack() as ctx:
        _emit_body(ctx, tc, ftg, lt, ltT, ltflat, aux, auxinit, lhaux, emitv,
                   cmat, out)
    nc.finalize()
    return nc


def _emit_body(ctx, tc, ftg, lt, ltT, ltflat, aux, auxinit, lhaux, emitv, cmat,
               out):
    nc = tc.nc

    const = ctx.enter_context(tc.tile_pool(name="const", bufs=1))
    raws = ctx.enter_context(tc.tile_pool(name="raws", bufs=2))
    states = ctx.enter_context(tc.tile_pool(name="states", bufs=4))
    endp = ctx.enter_context(tc.tile_pool(name="endp", bufs=2))
    ps_pool = ctx.enter_context(tc.tile_pool(name="ps", bufs=2, space="PSUM"))
    zw_pool = ctx.enter_context(tc.tile_pool(name="zw", bufs=1, space="PSUM"))
    pair_pool = ctx.enter_context(tc.tile_pool(name="pairps", bufs=1, space="PSUM"))

    # ---------------- small constants ----------------
    ones11 = const.tile([1, 2], F32)
    nc.gpsimd.memset(ones11[:], 1.0)
    onescol = const.tile([T, 1], F32)
    nc.gpsimd.memset(onescol[:], 1.0)
    mbiasF = const.tile([T, 1], F32)
    nc.gpsimd.memset(mbiasF[:], -BIAS)
    mbiasB = const.tile([P, 1], F32)
    nc.gpsimd.memset(mbiasB[64:112, :], -BIAS)

    # SP/HWDGE queue: everything gating the first rounds, smallest first.
    # HWDGE serializes at ~650ns/DMA, so queue order is the prologue.
    lt_sb = const.tile([T, T], F32)
    nc.sync.dma_start(lt_sb[:], lt[:])
    rawf0 = raws.tile([T, 512], F32, tag="rawf0")
    nc.sync.dma_start(rawf0[:], ftg[:, 0:512])
    ltT_sb = const.tile([P, T], F32)   # partitions 64..111: same-start Exp
    nc.sync.dma_start(ltT_sb[64:112, :], ltT[:])
    rootcol = const.tile([T, 1], F32)
    nc.sync.dma_start(rootcol[:], lt[ROOT:ROOT + 1, :].rearrange("a b -> b a"))
    biasvec = const.tile([T, 1], F32)
    nc.vector.tensor_scalar_add(biasvec[:], rootcol[:], -BIAS)

    # ---------------- stationary matrices [112, 112] ----------------
    # col j (output partition): 0..47 fwd T^T; 48 w (tend); 49 const-1;
    # 50 carrier; 51 accumulator; 64..111 bwd T (+ inj via row 50).
    lhsT_init = const.tile([P, P], F32)
    lhsT_main = const.tile([P, P], F32)
    lhsT_fin = const.tile([P, P], F32)
    nc.gpsimd.memset(lhsT_init[:], 0.0)
    nc.gpsimd.memset(lhsT_main[:], 0.0)
    nc.gpsimd.memset(lhsT_fin[:], 0.0)
    # rows 48..51 (selector routing, const-1, tend injection row) are tiny
    # host-baked patterns: one DMA per tile
    lhts = (lhsT_init, lhsT_main, lhsT_fin)
    for k, lh in enumerate(lhts):
        nc.sync.dma_start(lh[48:52, :], lhaux[:, k * P:(k + 1) * P])
    # initial state: zeros except row 49 = 1 and row 50 = inject(L=256)
    st0 = states.tile([P, BL], F32, tag="st")
    nc.gpsimd.memset(st0[:], 0.0)
    nc.sync.dma_start(st0[49:51, :], auxinit[:])
    for lh in (lhsT_init, lhsT_main):
        nc.scalar.activation(lh[0:T, 0:T], lt_sb[:], AF.Exp)          # A block
        nc.scalar.activation(lh[0:T, 48:49], lt_sb[:, END:END + 1], AF.Exp)  # w col
    for lh in lhts:
        nc.scalar.activation(lh[64:112, 64:112], ltT_sb[64:112, :], AF.Exp)  # B block

    # ---------------- E slab [112, 128*64] ----------------
    # fwd rows 0..47 (t = round) via SP/HWDGE, bwd rows 64..111
    # (tau = 255 - round, host pre-reversed) via Pool/SWDGE — parallel DMA
    # paths, fwd/bwd interleaved so round k's blocks arrive in step.
    slab = const.tile([P, R * BL], F32)
    pieces = [(0, 512), (512, 512), (1024, 1024), (2048, 2048), (4096, 4096)]
    chunks = [(0, 512), (512, 1536), (2048, 2048), (4096, 4096)]
    rawb0 = raws.tile([P, 512], F32, tag="rawb0")
    nc.gpsimd.dma_start(rawb0[64:112, :], ftg[:, 8192:8704])
    nc.gpsimd.memset(slab[32:64, 0:512], 0.0)
    nc.gpsimd.dma_start(slab[48:52, 0:512], aux[:, 0:512])
    for pi, (off, ln) in enumerate(pieces):
        if off == 0:
            # block 0 carries the ROOT prior in its bias; remainder plain
            nc.scalar.activation(slab[0:T, 0:BL], rawf0[:, 0:BL], AF.Exp,
                                 bias=biasvec[:])
            nc.scalar.activation(slab[0:T, BL:ln], rawf0[:, BL:ln], AF.Exp,
                                 bias=mbiasF[:])
            nc.scalar.activation(slab[64:112, off:off + ln], rawb0[64:112, :],
                                 AF.Exp, bias=mbiasB[64:112, :])
            continue
        rawf = raws.tile([T, 4096], F32, tag="rawf")
        nc.sync.dma_start(rawf[:, 0:ln], ftg[:, off:off + ln])
        rawb = raws.tile([P, 4096], F32, tag="rawb")
        nc.gpsimd.dma_start(rawb[64:112, 0:ln], ftg[:, 8192 + off:8192 + off + ln])
        nc.scalar.activation(slab[0:T, off:off + ln], rawf[:, 0:ln], AF.Exp,
                             bias=mbiasF[:])
        nc.scalar.activation(slab[64:112, off:off + ln], rawb[64:112, 0:ln],
                             AF.Exp, bias=mbiasB[64:112, :])
        if pi + 1 < len(chunks):
            coff, cln = chunks[pi + 1]
            nc.gpsimd.memset(slab[32:64, coff:coff + cln], 0.0)
            nc.gpsimd.dma_start(slab[48:52, coff:coff + cln], aux[:, coff:coff + cln])

    # late constants (gold score path) on the Pool/SWDGE queue
    ltsb = const.tile([128, 18], F32)
    nc.gpsimd.dma_start(ltsb[:], ltflat[:])
    cmat_sb = const.tile([128, 18 * BL], F32)
    nc.gpsimd.dma_start(cmat_sb[:], cmat[:])
    emitv_sb = const.tile([BL, M], F32)
    nc.gpsimd.dma_start(emitv_sb[:], emitv[:])

    # prefetch the Ln act table once the Exps are done (reads last exp'd cell
    # to order after them on the Act queue)
    lnjunk = const.tile([1, 1], F32)
    nc.scalar.activation(lnjunk[:], slab[0:1, R * BL - 1:R * BL], AF.Ln)

    # ---------------- gold score (independent of the scan) ----------------
    emitsum = const.tile([BL, 1], F32)
    nc.vector.tensor_reduce(emitsum[:], emitv_sb[:], axis=mybir.AxisListType.X,
                            op=ALU.add)
    pair_ps = pair_pool.tile([BL, 1], F32, space="PSUM")
    for k in range(18):
        nc.tensor.matmul(
            out=pair_ps[:],
            lhsT=cmat_sb[:, k * BL:(k + 1) * BL],
            rhs=ltsb[:, k:k + 1],
            start=(k == 0),
            stop=(k == 17),
        )

    # ---------------- the scan: 128 rounds + final matmul ----------------
    prev = st0
    ps = None
    ps128 = None
    for i in range(R + 1):
        col = i % 8
        if col == 0:
            ps = ps_pool.tile([P, 512], F32, space="PSUM")
        lh = lhsT_init if i == 0 else (lhsT_fin if i == R else lhsT_main)
        nc.tensor.matmul(out=ps[:, col * BL:(col + 1) * BL], lhsT=lh[:],
                         rhs=prev[:], start=True, stop=True)
        if i < R:
            cur = states.tile([P, BL], F32, tag="st")
            eng = nc.gpsimd if os.environ.get("K_POOL_TT") else nc.vector
            eng.tensor_tensor(out=cur[:], in0=ps[:, col * BL:(col + 1) * BL],
                              in1=slab[:, i * BL:(i + 1) * BL], op=ALU.mult)
            prev = cur
        else:
            ps128 = ps[:, col * BL:(col + 1) * BL]

    # ---------------- epilogue ----------------
    # z (long sequences): zz = p_127 o beta_127, then column-sum via matmul.
    zz = endp.tile([T, BL], F32)
    nc.vector.tensor_tensor(out=zz[:], in0=prev[0:T, :], in1=ps128[64:112],
                            op=ALU.mult)
    # wsel (short sequences): accumulated at psum row 32 by lhsT_fin.
    wrow = endp.tile([1, BL], F32)
    nc.vector.tensor_copy(wrow[:], ps128[32:33])
    # part0[b] = z[b] + wsel[b] (disjoint support), as a [64,1] column
    zw = zw_pool.tile([BL, 1], F32, space="PSUM")
    nc.tensor.matmul(out=zw[:], lhsT=zz[:], rhs=onescol[:], start=True, stop=False)
    nc.tensor.matmul(out=zw[:], lhsT=wrow[:], rhs=ones11[0:1, 0:1], start=False,
                     stop=True)
    zws = endp.tile([BL, 1], F32)
    nc.vector.tensor_scalar(out=zws[:], in0=zw[:], scalar1=1e-37,
                            scalar2=2.0 ** -12, op0=ALU.max, op1=ALU.mult)
    lnp = endp.tile([BL, 1], F32)
    nc.scalar.activation(lnp[:], zws[:], AF.Ln)
    # nll = ln(part0) - pair - emitsum   (scale/BIAS comp folded into emitv)
    n1 = endp.tile([BL, 1], F32)
    nc.vector.tensor_tensor(out=n1[:], in0=lnp[:], in1=pair_ps[:], op=ALU.subtract)
    nll = endp.tile([BL, 1], F32)
    nc.vector.tensor_tensor(out=nll[:], in0=n1[:], in1=emitsum[:], op=ALU.subtract)
    nc.sync.dma_start(out[:], nll[:])


# ---------------- host side ----------------

def _host_prep_core(feats_c, tags_c, lengths_c):
    """Host work: layout transforms + integer index/mask/count prep +
    emission value selection (no float arithmetic beyond constant offsets)."""
    L = lengths_c.astype(np.int64)
    tg = tags_c.astype(np.int64)
    bidx = np.arange(BL)
    tidx = np.arange(M)

    ft = np.ascontiguousarray(feats_c.transpose(2, 1, 0))          # [48, 256, 64]
    ftg = np.empty((T, 2 * R * BL), np.float32)
    ftg[:, :R * BL] = ft[:, 0:R, :].reshape(T, R * BL)
    ftg[:, R * BL:] = ft[:, :R - 1:-1, :].reshape(T, R * BL)       # tau = 255-i

    rr = np.arange(R)
    aux = np.zeros((4, R, BL), np.float32)
    aux[0] = (L[None, :] == rr[:, None])           # row 48: select w at r == L
    aux[1] = 1.0                                   # row 49: const one
    aux[2] = (L[None, :] == 255 - rr[:, None])     # row 50: inject at L == 255-r
    aux[3] = 1.0                                   # row 51: accumulator carry

    auxinit = np.zeros((2, BL), np.float32)
    auxinit[0] = 1.0
    auxinit[1] = (L == 256)

    mask = tidx[None, :] < L[:, None]
    emitv = np.where(mask, feats_c[bidx[:, None], tidx[None, :], tg], 0.0)
    emitv = np.ascontiguousarray(emitv, np.float32)
    # fold the exact renorm/Ln-scale compensation into the emission sum
    emitv[:, 0] -= (L * BIAS + LN2_12).astype(np.float32)

    cfull = np.zeros((T * T, BL), np.float32)
    prev = tg[:, :-1]
    nxt = tg[:, 1:]
    pmask = (tidx[1:][None, :] < L[:, None])
    pidx = (prev * T + nxt)
    np.add.at(cfull, (pidx[pmask], np.broadcast_to(bidx[:, None], pidx.shape)[pmask]), 1.0)
    np.add.at(cfull, (ROOT * T + tg[:, 0], bidx), 1.0)
    last = tg[bidx, L - 1]
    np.add.at(cfull, (last * T + END, bidx), 1.0)
    cmat = np.ascontiguousarray(
        cfull.reshape(18, 128, BL).transpose(1, 0, 2)).reshape(128, 18 * BL)

    return {
        "ftg": ftg,
        "aux": aux.reshape(4, R * BL),
        "auxinit": auxinit,
        "emitv": emitv,
        "cmat": cmat,
    }


def kernel(feats, tags, lengths, log_transitions):
    global _PROGRAM
    feats = np.asarray(feats, np.float32)
    tags = np.asarray(tags)
    lengths = np.asarray(lengths)
    lt = np.asarray(log_transitions, np.float32)
    ltT = np.ascontiguousarray(lt.T)
    ltflat = np.ascontiguousarray(lt.reshape(-1).reshape(18, 128).T)

    # rows 48..51 of the three stationary matrices (48=w->acc routing,
    # 49=const-1 + carrier feed, 50=tend injection, 51=acc carry)
    tend = np.exp(lt[:, END].astype(np.float64)).astype(np.float32)
    lhaux = np.zeros((4, 3, P), np.float32)
    for k in range(3):
        lhaux[1, k, 49] = 1.0   # psum[49] = state[49] (const 1)
        lhaux[1, k, 50] = 1.0   # psum[50] = state[49] (carrier feed)
        lhaux[2, k, 64:112] = tend   # injection into bwd block
    lhaux[1, 0, 0:T] = 1.0      # init: seed fwd state with ones
    lhaux[0, 1, 51] = 1.0       # main: acc += selected w
    lhaux[3, 1, 51] = 1.0       # main: acc carry
    lhaux[0, 2, 32] = 1.0       # fin: route (sel + acc) to psum row 32
    lhaux[3, 2, 32] = 1.0
    lhaux = lhaux.reshape(4, 3 * P)

    in_maps = []
    for c in range(NC):
        sl = slice(c * BL, (c + 1) * BL)
        m = _host_prep_core(feats[sl], tags[sl], lengths[sl])
        m["lt"] = lt
        m["ltT"] = ltT
        m["ltflat"] = ltflat
        m["lhaux"] = lhaux
        in_maps.append(m)

    if _PROGRAM is None:
        _PROGRAM = _build_program()

    res = run_bass_kernel_spmd(_PROGRAM, in_maps, core_ids=list(range(NC)))
    return np.concatenate([r["out"].reshape(BL) for r in res.results])


if __name__ == "__main__":
    rng = np.random.default_rng(0)
    feats = rng.standard_normal((B, M, T)).astype(np.float32)
    tags = rng.integers(0, ROOT, (B, M)).astype(np.int32)
    lengths = rng.integers(1, M + 1, (B,)).astype(np.int32)
    std = (2.0 / (T + T)) ** 0.5
    lt = (rng.standard_normal((T, T)) * std).astype(np.float32)
    lt[:, ROOT] = -10000.0
    lt[END, :] = -10000.0
    out = kernel(feats, tags, lengths, lt)
    print(out[:8], out.shape, out.dtype)
